# revision 1
# baseline (speedup 1.0000x reference)
"""Trainium2 Bass kernel for the attention-LSTM caption decoder (8 cores).

Sharding: gate-dimension sharding of the recurrence (core k owns D-rows
[k*128,(k+1)*128) of each i/f/g/o gate block; weights SBUF-resident fp32r;
full batch B=256 on the matmul moving dim). Attention is batch-sharded
(32 rows/core); dec_att comes from a partial GEMM + ReduceScatter so the
core-relative shard falls out of the collective (program stays SPMD-uniform).
aw/h1/h2 are exchanged via small intra-chip AllGathers. The vocab projection
is deferred: gathered h2 history feeds one big fp32r GEMM, vocab-sharded.

at_b is omitted deliberately: softmax(x + c) == softmax(x) exactly.
ad_b is folded into the img_att bias (af_b + ad_b) so the ReduceScatter
doesn't multiply it by 8.
"""

import numpy as np
import ml_dtypes

import concourse.bass as bass
import concourse.bacc as bacc
import concourse.tile as tile
from concourse import mybir
from concourse.bass_utils import run_bass_kernel_spmd

F32 = mybir.dt.float32
F32R = mybir.dt.float32r
BF16 = mybir.dt.bfloat16
U8 = mybir.dt.uint8
AF = mybir.ActivationFunctionType

NC = 8
B, R, F = 256, 36, 2048
A, E, D, V = 512, 1024, 1024, 10000
L, T = 20, 19
BSH = B // NC            # 32
GS = 4 * (D // NC)       # 512 gate rows per core
VSH = 1280               # padded vocab shard
RB = R * BSH             # 1152
RG = [list(range(NC))]

_CACHED = {}


def bcast_r(ap, n):
    dims = list(ap.ap)
    dims.insert(1, [0, n])
    return bass.AP(tensor=ap.tensor, offset=ap.offset, ap=dims)


def mk_ap(ap, offset_elems, dims):
    return bass.AP(tensor=ap.tensor, offset=ap.offset + offset_elems, ap=dims)


def build(nsteps=T, debug=False):
    nc = bacc.Bacc(None, target_bir_lowering=False, debug=False)

    def inp(name, shape, dt):
        return nc.dram_tensor(name, shape, dt, kind="ExternalInput")

    wtdh2 = inp("wtdh2", [D, GS], F32R)
    wtdh1 = inp("wtdh1", [D, GS], F32R)
    wfavg = inp("wfavg", [F, GS], F32R)
    we = inp("we", [E, GS], F32R)
    tdb = inp("tdb", [1, GS], F32R)
    wlgaw = inp("wlgaw", [F, GS], F32R)
    wlgh1 = inp("wlgh1", [D, GS], F32R)
    wlgh2 = inp("wlgh2", [D, GS], F32R)
    lgb = inp("lgb", [1, GS], F32R)
    adwT = inp("adwT", [D, A], F32R)
    bsel = inp("bsel", [B, BSH], F32R)
    eye32 = inp("eye32", [BSH, BSH], F32R)
    afw = inp("afw", [F, A], BF16)
    afb = inp("afb", [1, A], BF16)          # af_b + ad_b
    atw = inp("atw", [A, 1], BF16)
    ftsT = inp("ftsT", [F, RB], BF16)
    ftsrb = inp("ftsrb", [RB, F], BF16)
    embsT = inp("embsT", [E, T * B], F32R)
    outw = inp("outw", [D, VSH], F32R)
    outb = inp("outb", [VSH, 1], F32)
    masks = inp("masks", [T, B], U8)
    maskf = inp("maskf", [T * B, 1], F32)
    bdiag = inp("bdiag", [128, BSH], BF16)
    t128 = inp("t128", [128, 128], BF16)

    outp = nc.dram_tensor("outp", [VSH, T * B], F32, kind="ExternalOutput")
    dbg = {}
    if debug:
        for nm, shp, dt_ in [("d_img", [128, 4, RB], BF16),
                             ("d_favgp", [128, 4, B], F32),
                             ("d_h1g", [128, 8, B], F32),
                             ("d_h2g", [128, 8, B], F32),
                             ("d_awg", [128, 16, B], F32),
                             ("d_dec", [128, 4, BSH], BF16),
                             ("d_exps", [128, 9], F32),
                             ("d_arhs", [128, 9, BSH], BF16),
                             ("d_tdps", [128, 4, B], BF16),
                             ("d_favt", [128, 16, BSH], F32),
                             ("d_favgg", [128, 16, B], F32)]:
            dbg[nm] = nc.dram_tensor(nm, shp, dt_, kind="ExternalOutput")

    def dram(name, shape, dt, shared=False):
        return nc.dram_tensor(name, shape, dt,
                              addr_space="Shared" if shared else "Local")

    agin_favg = dram("agin_favg", [F, BSH], F32)
    agout_favg = dram("agout_favg", [NC, F, BSH], F32, True)
    agin_aw = dram("agin_aw", [F, BSH], F32)
    agout_aw = dram("agout_aw", [NC, F, BSH], F32, True)
    agin_h1 = dram("agin_h1", [128, B], F32)
    agout_h1 = dram("agout_h1", [NC, 128, B], F32, True)
    agin_h2 = dram("agin_h2", [128, B], F32)
    agout_h2 = dram("agout_h2", [NC, 128, B], F32, True)
    h2hist = dram("h2hist", [T, D, B], F32)

    def ag(in_ap, out_ap):
        nc.gpsimd.collective_compute(
            "AllGather", mybir.AluOpType.bypass, replica_groups=RG,
            ins=[in_ap], outs=[out_ap])

    with tile.TileContext(nc) as tc:
        with (
            tc.tile_pool(name="wp", bufs=1) as wp,
            tc.tile_pool(name="stp", bufs=1) as stp,
        ):
            def load_T(pool, src, kdim, name):
                t = pool.tile([128, kdim // 128, GS], F32R, tag=name)
                for kt in range(kdim // 128):
                    nc.sync.dma_start(
                        out=t[:, kt, :],
                        in_=mk_ap(src[:, :], kt * 128 * GS,
                                  [[GS, 128], [1, GS]]))
                return t

            wtdh2_s = load_T(wp, wtdh2, D, "wtdh2")
            wtdh1_s = load_T(wp, wtdh1, D, "wtdh1")
            we_s = load_T(wp, we, E, "we")
            wlgaw_s = load_T(wp, wlgaw, F, "wlgaw")
            wlgh1_s = load_T(wp, wlgh1, D, "wlgh1")
            wlgh2_s = load_T(wp, wlgh2, D, "wlgh2")
            tdb_s = wp.tile([128, 4], F32, tag="tdb")
            nc.sync.dma_start(out=tdb_s,
                              in_=mk_ap(tdb[:, :].bitcast(F32), 0,
                                        [[1, 128], [128, 4]]))
            lgb_s = wp.tile([128, 4], F32, tag="lgb")
            nc.sync.dma_start(out=lgb_s,
                              in_=mk_ap(lgb[:, :].bitcast(F32), 0,
                                        [[1, 128], [128, 4]]))
            bsel_s = wp.tile([128, 2, BSH], F32R, tag="bsel")
            nc.sync.dma_start(out=bsel_s,
                              in_=bsel[:, :].rearrange("(c p) j -> p c j", p=128))
            eye32_s = wp.tile([BSH, BSH], F32R, tag="eye32")
            nc.sync.dma_start(out=eye32_s, in_=eye32[:, :])
            atw_s = wp.tile([128, 4], BF16, tag="atw")
            nc.sync.dma_start(out=atw_s,
                              in_=mk_ap(atw[:, :], 0, [[1, 128], [128, 4]]))
            bdiag_s = wp.tile([128, BSH], BF16, tag="bdiag")
            nc.sync.dma_start(out=bdiag_s, in_=bdiag[:, :])
            t128_s = wp.tile([128, 128], BF16, tag="t128")
            nc.sync.dma_start(out=t128_s, in_=t128[:, :])
            ones_b = wp.tile([1, 384], BF16, tag="ones_b")
            nc.vector.memset(ones_b, 1.0)
            img_s = wp.tile([128, 4, RB], BF16, tag="img")
            favgp_s = wp.tile([128, 4, B], F32, tag="favgp")

            h1_s = stp.tile([128, B], F32, tag="h1")
            c1_s = stp.tile([128, B], F32, tag="c1")
            h2_s = stp.tile([128, B], F32, tag="h2")
            c2_s = stp.tile([128, B], F32, tag="c2")
            for s in (h1_s, c1_s, h2_s, c2_s):
                nc.vector.memset(s, 0.0)
            h1g_s = stp.tile([128, 8, B], F32R, tag="h1g")
            h2g_s = stp.tile([128, 8, B], F32R, tag="h2g")
            nc.vector.memset(h1g_s.bitcast(F32), 0.0)
            nc.vector.memset(h2g_s.bitcast(F32), 0.0)

            # ================= phase 0 =================
            with (
                tc.tile_pool(name="p0", bufs=2) as p0,
                tc.tile_pool(name="p0w", bufs=1) as p0w,
                tc.tile_pool(name="p0ps", bufs=1, space="PSUM") as p0ps,
            ):
                afb_t = p0.tile([1, A], BF16, tag="afb")
                nc.sync.dma_start(out=afb_t, in_=afb[:, :])
                # img_att = af_w @ fts^T + (af_b + ad_b); streamed over kt
                for nch in range(3):
                    ia_ps = p0ps.tile([128, 4, 512], F32, tag="iaps")
                    for kt in range(16):
                        afw_c = p0.tile([128, A], BF16, tag="afwc")
                        nc.sync.dma_start(
                            out=afw_c,
                            in_=mk_ap(afw[:, :], kt * 128 * A,
                                      [[A, 128], [1, A]]))
                        fts_c = p0.tile([128, 384], BF16, tag="ftsc")
                        nc.sync.dma_start(
                            out=fts_c,
                            in_=mk_ap(ftsT[:, :], kt * 128 * RB + nch * 384,
                                      [[RB, 128], [1, 384]]))
                        for m in range(4):
                            nc.tensor.matmul(
                                ia_ps[:, m, 0:384],
                                afw_c[:, m * 128:(m + 1) * 128],
                                fts_c[:, :], start=(kt == 0), stop=False)
                    for m in range(4):
                        nc.tensor.matmul(
                            ia_ps[:, m, 0:384], afb_t[:, m * 128:(m + 1) * 128],
                            ones_b[:, :], start=False, stop=True)
                        nc.scalar.copy(
                            img_s[:, m, nch * 384:(nch + 1) * 384],
                            ia_ps[:, m, 0:384])
                # favg for my batch shard (mean over regions)
                favg_t = p0.tile([128, 16, BSH], F32, tag="favg")
                for kt in range(16):
                    fts_f = p0.tile([128, RB], BF16, tag="ftsf")
                    nc.sync.dma_start(
                        out=fts_f,
                        in_=mk_ap(ftsT[:, :], kt * 128 * RB,
                                  [[RB, 128], [1, RB]]))
                    nc.vector.reduce_sum(
                        out=favg_t[:, kt, :],
                        in_=fts_f.rearrange("p (r b) -> p b r", r=R),
                        axis=mybir.AxisListType.X)
                nc.scalar.mul(favg_t[:, :, :], favg_t[:, :, :], 1.0 / R)
                nc.sync.dma_start(
                    out=mk_ap(agin_favg[:, :], 0,
                              [[BSH, 128], [128 * BSH, 16], [1, BSH]]),
                    in_=favg_t)
                ag(agin_favg[:, :], agout_favg[:, :, :])
                favgg = p0w.tile([128, 16, B], F32R, tag="favgg")
                for r in range(NC):
                    nc.sync.dma_start(
                        out=favgg[:, :, r * BSH:(r + 1) * BSH],
                        in_=mk_ap(agout_favg[:, :, :].bitcast(F32R),
                                  r * F * BSH,
                                  [[BSH, 128], [128 * BSH, 16], [1, BSH]]))
                if debug:
                    nc.sync.dma_start(out=dbg["d_favt"].ap(), in_=favg_t)
                    nc.sync.dma_start(
                        out=dbg["d_favgg"].ap().bitcast(F32R), in_=favgg)
                fp_ps = p0ps.tile([128, 4, 512], F32, tag="fpps")
                for kt in range(16):
                    wfavg_c = p0.tile([128, GS], F32R, tag="wfavgc")
                    nc.sync.dma_start(
                        out=wfavg_c,
                        in_=mk_ap(wfavg[:, :], kt * 128 * GS,
                                  [[GS, 128], [1, GS]]))
                    for m in range(4):
                        nc.tensor.matmul(
                            fp_ps[:, m, 0:B],
                            wfavg_c[:, m * 128:(m + 1) * 128],
                            favgg[:, kt, :], start=(kt == 0), stop=(kt == 15))
                for m in range(4):
                    nc.vector.tensor_scalar_add(favgp_s[:, m, :],
                                                fp_ps[:, m, 0:B],
                                                tdb_s[:, m:m + 1])

            # ================= phase 1: recurrence =================
            with (
                tc.tile_pool(name="p1", bufs=2) as p1,
                tc.tile_pool(name="pawg", bufs=1) as pawg,
                tc.tile_pool(name="pemb", bufs=1) as pemb,
                tc.tile_pool(name="p1c", bufs=1) as p1c,
                tc.tile_pool(name="psg", bufs=1, space="PSUM") as psg,
                tc.tile_pool(name="psa", bufs=2, space="PSUM") as psa,
            ):
                dbg_t = {}
                for t in range(nsteps):
                    emb_t = pemb.tile([128, 8, B], F32R, tag="embt")
                    nc.sync.dma_start(
                        out=emb_t,
                        in_=mk_ap(embsT[:, :], t * B,
                                  [[T * B, 128], [128 * T * B, 8], [1, B]]))
                    mask_t = p1.tile([128, B], U8, tag="maskt")
                    mrow = masks[t:t + 1, :]
                    nc.sync.dma_start(
                        out=mask_t,
                        in_=bass.AP(tensor=mrow.tensor, offset=mrow.offset,
                                    ap=[[0, 128], [1, B]]))

                    # ---- td-LSTM gates (m-interleaved, bank-exclusive) ----
                    tdps = psg.tile([128, 4, 512], F32, tag="gps")
                    for m in range(4):
                        sl = slice(m * 128, (m + 1) * 128)
                        for kt in range(8):
                            nc.tensor.matmul(tdps[:, m, 0:B],
                                             wtdh1_s[:, kt, sl],
                                             h1g_s[:, kt, :],
                                             start=(kt == 0), stop=False)
                        for kt in range(8):
                            nc.tensor.matmul(tdps[:, m, 0:B], we_s[:, kt, sl],
                                             emb_t[:, kt, :],
                                             start=False, stop=False)
                    for m in range(4):
                        sl = slice(m * 128, (m + 1) * 128)
                        for kt in range(8):
                            nc.tensor.matmul(tdps[:, m, 0:B],
                                             wtdh2_s[:, kt, sl],
                                             h2g_s[:, kt, :],
                                             start=False, stop=(kt == 7))
                    if debug and t == 0:
                        dtd = p1c.tile([128, 4, B], BF16, tag="dapsb")
                        nc.vector.tensor_copy(dtd, tdps[:, :, 0:B])
                        nc.sync.dma_start(out=dbg["d_tdps"][:, :, :], in_=dtd)
                    tdv = tdps[:, :, 0:B]
                    nc.vector.tensor_add(tdv, tdv, favgp_s)
                    gi = p1c.tile([128, B], F32, tag="g0")
                    nc.scalar.activation(gi, tdps[:, 0, 0:B], AF.Sigmoid)
                    gf = p1c.tile([128, B], F32, tag="g1")
                    nc.scalar.activation(gf, tdps[:, 1, 0:B], AF.Sigmoid)
                    gg = p1c.tile([128, B], F32, tag="g2")
                    nc.scalar.activation(gg, tdps[:, 2, 0:B], AF.Tanh)
                    go = p1c.tile([128, B], F32, tag="g3")
                    nc.scalar.activation(go, tdps[:, 3, 0:B], AF.Sigmoid)
                    t1 = p1c.tile([128, B], F32, tag="x")
                    nc.vector.tensor_mul(t1, gf, c1_s)
                    t2 = p1c.tile([128, B], F32, tag="y")
                    nc.vector.tensor_mul(t2, gi, gg)
                    c1n = p1c.tile([128, B], F32, tag="g0")
                    nc.vector.tensor_add(c1n, t1, t2)
                    tc1 = p1c.tile([128, B], F32, tag="g1")
                    nc.scalar.activation(tc1, c1n, AF.Tanh)
                    h1n = p1c.tile([128, B], F32, tag="g2")
                    nc.vector.tensor_mul(h1n, go, tc1)
                    nc.vector.copy_predicated(c1_s, mask_t, c1n)
                    nc.vector.copy_predicated(h1_s, mask_t, h1n)

                    # ---- h1 allgather (lands during attention) ----
                    nc.sync.dma_start(out=agin_h1[:, :], in_=h1_s)
                    ag(agin_h1[:, :], agout_h1[:, :, :])
                    nc.sync.dma_start(
                        out=h1g_s,
                        in_=agout_h1[:, :, :].bitcast(F32R).rearrange(
                            "r p b -> p r b"))

                    # ---- dec_att via transposed GEMM + input-driven select
                    dtp = psg.tile([128, 2, 512], F32, tag="gps")
                    for kt in range(8):
                        adw_c = p1.tile([128, A], F32R, tag="adwc")
                        nc.sync.dma_start(
                            out=adw_c,
                            in_=mk_ap(adwT[:, :], kt * 128 * A,
                                      [[A, 128], [1, A]]))
                        for bc in range(2):
                            nc.tensor.matmul(
                                dtp[:, bc, :],
                                h1g_s[:, kt, bc * 128:(bc + 1) * 128],
                                adw_c[:, :], start=(kt == 0), stop=(kt == 7))
                    decT_sb = p1c.tile([128, 2, A], F32R, tag="decT")
                    nc.scalar.copy(decT_sb, dtp)
                    dsp = psa.tile([BSH, A], F32, tag="small")
                    for bc in range(2):
                        nc.tensor.matmul(dsp[:, :], bsel_s[:, bc, :],
                                         decT_sb[:, bc, :],
                                         start=(bc == 0), stop=(bc == 1))
                    dsel_sb = p1c.tile([BSH, A], F32R, tag="dsel")
                    nc.scalar.copy(dsel_sb, dsp)
                    ttp = psa.tile([128, 4, BSH], F32, tag="small")
                    for q in range(4):
                        nc.tensor.transpose(
                            ttp[:, q, :].bitcast(F32R),
                            dsel_sb[:, q * 128:(q + 1) * 128],
                            eye32_s[:, :])
                    dec = p1c.tile([128, 4, BSH], BF16, tag="dec")
                    nc.scalar.copy(dec, ttp)

                    # ---- attention ----
                    scps = psa.tile([128, 40], F32, tag="small")
                    for m in range(4):
                        rel = p1c.tile([128, RB], BF16, tag="rel")
                        nc.vector.tensor_add(
                            rel.rearrange("p (r b) -> p r b", r=R),
                            img_s[:, m, :].rearrange("p (r b) -> p r b", r=R),
                            bcast_r(dec[:, m, :], R))
                        nc.scalar.activation(rel, rel, AF.Relu)
                        for c in range(9):
                            nc.tensor.matmul(
                                scps[:, m * 9 + c:m * 9 + c + 1],
                                rel[:, c * 128:(c + 1) * 128],
                                atw_s[:, m:m + 1],
                                start=True, stop=True)
                    scs = p1c.tile([128, 9], F32, tag="scs")
                    nc.vector.reduce_sum(
                        out=scs,
                        in_=scps[:, 0:36].rearrange("p (m c) -> p c m", m=4),
                        axis=mybir.AxisListType.X)
                    exps = p1c.tile([128, 9], F32, tag="exps")
                    nc.scalar.activation(exps, scs, AF.Exp)
                    expb = p1c.tile([128, 9], BF16, tag="expb")
                    nc.vector.tensor_copy(expb, exps)
                    for c in range(9):
                        nc.tensor.matmul(scps[:, 36:37], t128_s[:, :],
                                         expb[:, c:c + 1],
                                         start=(c == 0), stop=(c == 8))
                    rinv = p1c.tile([128, 1], F32, tag="rinv")
                    nc.vector.reciprocal(rinv, scps[:, 36:37])
                    arhs = p1c.tile([128, 9, BSH], BF16, tag="arhs")
                    for c in range(9):
                        nc.vector.tensor_scalar(
                            arhs[:, c, :], bdiag_s, exps[:, c:c + 1],
                            rinv[:, 0:1], mybir.AluOpType.mult,
                            mybir.AluOpType.mult)
                    # ---- aw einsum (fts chunks streamed) ----
                    awacc = p1c.tile([128, 16, BSH], F32, tag="awsb")
                    for c in range(9):
                        fts_e = p1.tile([128, F], BF16, tag="ftse")
                        nc.sync.dma_start(
                            out=fts_e,
                            in_=mk_ap(ftsrb[:, :], c * 128 * F,
                                      [[F, 128], [1, F]]))
                        awps = psa.tile([128, 16, BSH], F32, tag="awps")
                        for fc in range(16):
                            nc.tensor.matmul(
                                awps[:, fc, :],
                                fts_e[:, fc * 128:(fc + 1) * 128],
                                arhs[:, c, :],
                                start=True, stop=True)
                        if c == 0:
                            nc.vector.tensor_copy(awacc, awps)
                        else:
                            nc.vector.tensor_add(awacc, awacc, awps)
                    nc.sync.dma_start(
                        out=mk_ap(agin_aw[:, :], 0,
                                  [[BSH, 128], [128 * BSH, 16], [1, BSH]]),
                        in_=awacc)
                    ag(agin_aw[:, :], agout_aw[:, :, :])
                    awg = pawg.tile([128, 16, B], F32R, tag="awg")
                    for r in range(NC):
                        nc.sync.dma_start(
                            out=awg[:, :, r * BSH:(r + 1) * BSH],
                            in_=mk_ap(agout_aw[:, :, :].bitcast(F32R),
                                      r * F * BSH,
                                      [[BSH, 128], [128 * BSH, 16], [1, BSH]]))

                    if debug and t == 0:
                        for nm, tl in [("d_dec", dec), ("d_exps", exps),
                                       ("d_arhs", arhs), ("d_awg", awg)]:
                            nc.sync.dma_start(
                                out=dbg[nm].ap().bitcast(tl.dtype), in_=tl)
                    # ---- lg-LSTM gates ----
                    lgps = psg.tile([128, 4, 512], F32, tag="gps")
                    for m in range(4):
                        sl = slice(m * 128, (m + 1) * 128)
                        for kt in range(8):
                            nc.tensor.matmul(lgps[:, m, 0:B],
                                             wlgh1_s[:, kt, sl],
                                             h1g_s[:, kt, :],
                                             start=(kt == 0), stop=False)
                        for kt in range(8):
                            nc.tensor.matmul(lgps[:, m, 0:B],
                                             wlgh2_s[:, kt, sl],
                                             h2g_s[:, kt, :],
                                             start=False, stop=False)
                    for m in range(4):
                        sl = slice(m * 128, (m + 1) * 128)
                        for kt in range(16):
                            nc.tensor.matmul(lgps[:, m, 0:B],
                                             wlgaw_s[:, kt, sl],
                                             awg[:, kt, :],
                                             start=False, stop=(kt == 15))
                    gi2 = p1c.tile([128, B], F32, tag="g0")
                    nc.scalar.activation(gi2, lgps[:, 0, 0:B], AF.Sigmoid, bias=lgb_s[:, 0:1])
                    gf2 = p1c.tile([128, B], F32, tag="g1")
                    nc.scalar.activation(gf2, lgps[:, 1, 0:B], AF.Sigmoid, bias=lgb_s[:, 1:2])
                    gg2 = p1c.tile([128, B], F32, tag="g2")
                    nc.scalar.activation(gg2, lgps[:, 2, 0:B], AF.Tanh, bias=lgb_s[:, 2:3])
                    go2 = p1c.tile([128, B], F32, tag="g3")
                    nc.scalar.activation(go2, lgps[:, 3, 0:B], AF.Sigmoid, bias=lgb_s[:, 3:4])
                    t3 = p1c.tile([128, B], F32, tag="x")
                    nc.vector.tensor_mul(t3, gf2, c2_s)
                    t4 = p1c.tile([128, B], F32, tag="y")
                    nc.vector.tensor_mul(t4, gi2, gg2)
                    c2n = p1c.tile([128, B], F32, tag="g0")
                    nc.vector.tensor_add(c2n, t3, t4)
                    tc2 = p1c.tile([128, B], F32, tag="g1")
                    nc.scalar.activation(tc2, c2n, AF.Tanh)
                    h2n = p1c.tile([128, B], F32, tag="g2")
                    nc.vector.tensor_mul(h2n, go2, tc2)
                    nc.vector.copy_predicated(c2_s, mask_t, c2n)
                    nc.vector.copy_predicated(h2_s, mask_t, h2n)

                    # ---- h2 allgather + history ----
                    nc.sync.dma_start(out=agin_h2[:, :], in_=h2_s)
                    ag(agin_h2[:, :], agout_h2[:, :, :])
                    nc.sync.dma_start(
                        out=h2g_s,
                        in_=agout_h2[:, :, :].bitcast(F32R).rearrange(
                            "r p b -> p r b"))
                    nc.sync.dma_start(
                        out=h2hist[t, :, :],
                        in_=agout_h2[:, :, :].rearrange("r p b -> (r p) b"))
                    if debug and t == nsteps - 1:
                        for nm, tl in [("d_img", img_s), ("d_favgp", favgp_s),
                                       ("d_h1g", h1g_s), ("d_h2g", h2g_s)]:
                            nc.sync.dma_start(
                                out=dbg[nm].ap().bitcast(tl.dtype), in_=tl)

        # ================= phase 2: vocab projection =================
        with (
            tc.tile_pool(name="p2w", bufs=1) as p2w,
            tc.tile_pool(name="p2", bufs=2) as p2,
            tc.tile_pool(name="p2ps", bufs=4, space="PSUM") as p2ps,
        ):
            outw_t = p2w.tile([128, 8, VSH], F32R, tag="outw")
            for kt in range(8):
                nc.sync.dma_start(
                    out=outw_t[:, kt, :],
                    in_=mk_ap(outw[:, :], kt * 128 * VSH,
                              [[VSH, 128], [1, VSH]]))
            outb_t = p2w.tile([128, 10], F32, tag="outb")
            nc.sync.dma_start(
                out=outb_t, in_=mk_ap(outb[:, :], 0, [[1, 128], [128, 10]]))
            NCHUNK = [(i * 512, 512) for i in range(9)] + [(4608, 256)]
            for (off, n) in NCHUNK:
                h2r_t = p2.tile([128, 8, 512], F32R, tag="h2rt")
                nt = n // B
                t0 = off // B
                for tt in range(nt):
                    nc.sync.dma_start(
                        out=h2r_t[:, :, tt * B:(tt + 1) * B],
                        in_=mk_ap(h2hist[:, :, :].bitcast(F32R),
                                  (t0 + tt) * D * B,
                                  [[B, 128], [128 * B, 8], [1, B]]))
                mk_t = p2.tile([128, 512], F32, tag="mkt")
                mf = maskf[:, :]
                nc.sync.dma_start(
                    out=mk_t[:, 0:n],
                    in_=bass.AP(tensor=mf.tensor, offset=mf.offset + off,
                                ap=[[0, 128], [1, n]]))
                for m in range(10):
                    lps = p2ps.tile([128, 512], F32, tag="lps")
                    for kt in range(8):
                        nc.tensor.matmul(
                            lps[:, 0:n],
                            outw_t[:, kt, m * 128:(m + 1) * 128],
                            h2r_t[:, kt, 0:n],
                            start=(kt == 0), stop=(kt == 7))
                    ls = p2.tile([128, 512], F32, tag="ls")
                    nc.scalar.copy(ls[:, 0:n], lps[:, 0:n])
                    nc.vector.tensor_scalar_add(ls[:, 0:n], ls[:, 0:n],
                                                outb_t[:, m:m + 1])
                    nc.vector.tensor_mul(ls[:, 0:n], ls[:, 0:n], mk_t[:, 0:n])
                    nc.sync.dma_start(
                        out=mk_ap(outp[:, :], m * 128 * T * B + off,
                                  [[T * B, 128], [1, n]]),
                        in_=ls[:, 0:n])

    nc.compile()
    return nc


def _build_cached():
    if "nc" not in _CACHED:
        _CACHED["nc"] = build()
    return _CACHED["nc"]


def host_prep(feats, sequences, sizes, emb, td_wih, td_whh, td_b,
              lg_wih, lg_whh, lg_b, af_w, af_b, ad_w, ad_b, at_w, at_b,
              out_w, out_b):
    f32 = np.float32
    bf = ml_dtypes.bfloat16
    lens = np.asarray(sizes).astype(np.int64)[:, 0]
    order = np.argsort(-lens, kind="stable")
    lens_s = lens[order]
    seq = np.asarray(sequences).astype(np.int64)[order]
    fts = np.ascontiguousarray(np.asarray(feats, f32)[order])

    embs = np.asarray(emb, f32)[seq[:, :T]]
    embsT = np.ascontiguousarray(embs.transpose(2, 1, 0)).reshape(E, T * B)

    mask = (np.arange(T)[None, :] < (lens_s - 1)[:, None])
    masks = np.ascontiguousarray(mask.T).astype(np.uint8)
    maskf = np.ascontiguousarray(mask.T.reshape(T * B, 1)).astype(f32)

    bdiag = np.tile(np.eye(BSH, dtype=f32), (4, 1)).astype(bf)
    t128 = np.tile(np.eye(BSH, dtype=f32), (4, 4)).astype(bf)

    td_wih = np.asarray(td_wih, f32)
    td_whh = np.asarray(td_whh, f32)
    lg_wih = np.asarray(lg_wih, f32)
    lg_whh = np.asarray(lg_whh, f32)
    af_wT = np.ascontiguousarray(np.asarray(af_w, f32).T).astype(bf)
    afb_full = np.asarray(af_b, f32) + np.asarray(ad_b, f32)
    ad_wv = np.asarray(ad_w, f32)
    adwT_full = np.ascontiguousarray(ad_wv.T)
    eye32_np = np.eye(BSH, dtype=f32)

    def bsel_k(k):
        m = np.zeros((B, BSH), f32)
        m[np.arange(k * BSH, (k + 1) * BSH), np.arange(BSH)] = 1.0
        return m
    atwT = np.ascontiguousarray(np.asarray(at_w, f32).T).astype(bf)
    out_wv = np.asarray(out_w, f32)
    out_bv = np.asarray(out_b, f32)

    in_maps = []
    for k in range(NC):
        gsl = np.concatenate([np.arange(g * D + k * 128, g * D + (k + 1) * 128)
                              for g in range(4)])
        bsl = slice(k * BSH, (k + 1) * BSH)
        fsh = fts[bsl]
        ftsT_k = np.ascontiguousarray(
            fsh.transpose(2, 1, 0).reshape(F, RB)).astype(bf)
        ftsrb_k = np.ascontiguousarray(
            fsh.transpose(1, 0, 2).reshape(RB, F)).astype(bf)
        ow_pad = np.zeros((VSH, D), f32)
        ow_pad[:1250] = out_wv[k * 1250:(k + 1) * 1250]
        ob_pad = np.zeros((VSH, 1), f32)
        ob_pad[:1250, 0] = out_bv[k * 1250:(k + 1) * 1250]
        in_maps.append({
            "wtdh2": np.ascontiguousarray(td_wih[gsl, 0:D].T),
            "wtdh1": np.ascontiguousarray(td_whh[gsl].T),
            "wfavg": np.ascontiguousarray(td_wih[gsl, D:D + F].T),
            "we": np.ascontiguousarray(td_wih[gsl, D + F:].T),
            "tdb": np.ascontiguousarray(np.asarray(td_b, f32)[gsl][None, :]),
            "wlgaw": np.ascontiguousarray(lg_wih[gsl, 0:F].T),
            "wlgh1": np.ascontiguousarray(lg_wih[gsl, F:].T),
            "wlgh2": np.ascontiguousarray(lg_whh[gsl].T),
            "lgb": np.ascontiguousarray(np.asarray(lg_b, f32)[gsl][None, :]),
            "adwT": adwT_full,
            "bsel": bsel_k(k),
            "eye32": eye32_np,
            "afw": af_wT,
            "afb": np.ascontiguousarray(afb_full[None, :]).astype(bf),
            "atw": atwT,
            "ftsT": ftsT_k,
            "ftsrb": ftsrb_k,
            "embsT": embsT,
            "outw": np.ascontiguousarray(ow_pad.T),
            "outb": ob_pad,
            "masks": masks,
            "maskf": maskf,
            "bdiag": bdiag,
            "t128": t128,
        })
    return in_maps


def kernel(**inputs):
    in_maps = host_prep(**inputs)
    nc = _build_cached()
    res = run_bass_kernel_spmd(nc, in_maps, core_ids=list(range(NC)))
    shards = [res.results[k]["outp"].reshape(VSH, T, B)[:1250]
              for k in range(NC)]
    full = np.concatenate(shards, axis=0)
    return np.ascontiguousarray(full.transpose(2, 1, 0))



# revision 7
# speedup vs baseline: 1.8114x; 1.8114x over previous
"""Trainium2 Bass kernel for the attention-LSTM caption decoder (8 cores).

Sharding: gate-dimension sharding of the recurrence (core k owns D-rows
[k*128,(k+1)*128) of each i/f/g/o gate block; weights SBUF-resident bf16;
full batch B=256 on the matmul moving dim). Attention is batch-sharded
(32 rows/core). aw/h1/h2 are exchanged via small intra-chip AllGathers
(bf16 payloads). The vocab projection is interleaved into the recurrence
(vocab GEMM for step t-1 runs inside step t's h2-AllGather window),
vocab-sharded across cores.

at_b is omitted deliberately: softmax(x + c) == softmax(x) exactly.
ad_b is folded into the img_att bias (af_b + ad_b).

LSTM cell state (c1/c2) and h state stay f32 in SBUF; bf16 is used for
GEMM operands (weights stationary, h/emb/aw moving) and AG payloads.
"""

import numpy as np
import ml_dtypes

import concourse.bass as bass
import concourse.bacc as bacc
import concourse.tile as tile
from concourse import mybir
from concourse.bass_utils import run_bass_kernel_spmd

F32 = mybir.dt.float32
F32R = mybir.dt.float32r
BF16 = mybir.dt.bfloat16
U8 = mybir.dt.uint8
AF = mybir.ActivationFunctionType

NC = 8
B, R, F = 256, 36, 2048
A, E, D, V = 512, 1024, 1024, 10000
L, T = 20, 19
BSH = B // NC            # 32
GS = 4 * (D // NC)       # 512 gate rows per core
VSH = 1280               # padded vocab shard
RB = R * BSH             # 1152
TB = T * B
RG = [list(range(NC))]

_CACHED = {}


def bcast_r(ap, n):
    dims = list(ap.ap)
    dims.insert(1, [0, n])
    return bass.AP(tensor=ap.tensor, offset=ap.offset, ap=dims)


def mk_ap(ap, offset_elems, dims):
    return bass.AP(tensor=ap.tensor, offset=ap.offset + offset_elems, ap=dims)


def build(nsteps=T, debug=False, fake_ag=False):
    nc = bacc.Bacc(None, target_bir_lowering=False, debug=False)

    def inp(name, shape, dt):
        return nc.dram_tensor(name, shape, dt, kind="ExternalInput")

    wtdh2 = inp("wtdh2", [D, GS], BF16)
    wtdh1 = inp("wtdh1", [D, GS], BF16)
    wfavg = inp("wfavg", [F, GS], BF16)
    we = inp("we", [E, GS], BF16)
    tdb = inp("tdb", [1, GS], F32)
    wlgaw = inp("wlgaw", [F, GS], BF16)
    wlgh1 = inp("wlgh1", [D, GS], BF16)
    wlgh2 = inp("wlgh2", [D, GS], BF16)
    lgb = inp("lgb", [1, GS], F32)
    adwT = inp("adwT", [D, A], BF16)
    bsel = inp("bsel", [B, BSH], F32R)
    eye32 = inp("eye32", [BSH, BSH], F32R)
    afw = inp("afw", [F, A], BF16)
    afb = inp("afb", [1, A], BF16)          # af_b + ad_b
    atw = inp("atw", [A, 1], BF16)
    ftsT = inp("ftsT", [F, RB], BF16)
    ftsrb = inp("ftsrb", [RB, F], BF16)
    embsT = inp("embsT", [E, TB], BF16)
    outw = inp("outw", [D, VSH], BF16)
    outb = inp("outb", [VSH, 1], F32)
    masks = inp("masks", [T, B], U8)
    maskf = inp("maskf", [TB, 1], F32)
    bdiag = inp("bdiag", [128, BSH], BF16)
    t128 = inp("t128", [128, 128], BF16)

    outp = nc.dram_tensor("outp", [VSH, TB], F32, kind="ExternalOutput")

    def dram(name, shape, dt, shared=False):
        return nc.dram_tensor(name, shape, dt,
                              addr_space="Shared" if shared else "Local")

    agin_favg = dram("agin_favg", [F, BSH], BF16)
    agout_favg = dram("agout_favg", [NC, F, BSH], BF16, True)
    agin_aw = dram("agin_aw", [F, BSH], BF16)
    agout_aw = dram("agout_aw", [NC, F, BSH], BF16, True)
    agin_h1 = dram("agin_h1", [128, B], BF16)
    agout_h1 = dram("agout_h1", [NC, 128, B], BF16, True)
    agin_h2 = dram("agin_h2", [128, B], BF16)
    agout_h2 = dram("agout_h2", [NC, 128, B], BF16, True)

    def ag(in_ap, out_ap):
        if fake_ag:
            # timeline-sim stand-in: same DRAM traffic shape, no collective
            for r in range(NC):
                nc.sync.dma_start(out=out_ap[r], in_=in_ap)
            return
        nc.gpsimd.collective_compute(
            "AllGather", mybir.AluOpType.bypass, replica_groups=RG,
            ins=[in_ap], outs=[out_ap])

    with tile.TileContext(nc) as tc:
        with (
            tc.tile_pool(name="wp", bufs=1) as wp,
            tc.tile_pool(name="stp", bufs=1) as stp,
        ):
            def load_T(pool, src, kdim, cols, name, dt=BF16):
                ktn = kdim // 128
                t = pool.tile([128, ktn, cols], dt, tag=name)
                nc.sync.dma_start(
                    out=t,
                    in_=mk_ap(src[:, :], 0,
                              [[cols, 128], [128 * cols, ktn], [1, cols]]))
                return t

            wtdh2_s = load_T(wp, wtdh2, D, GS, "wtdh2")
            wtdh1_s = load_T(wp, wtdh1, D, GS, "wtdh1")
            we_s = load_T(wp, we, E, GS, "we")
            wlgaw_s = load_T(wp, wlgaw, F, GS, "wlgaw")
            wlgh1_s = load_T(wp, wlgh1, D, GS, "wlgh1")
            wlgh2_s = load_T(wp, wlgh2, D, GS, "wlgh2")
            tdb_s = wp.tile([128, 4], F32, tag="tdb")
            nc.sync.dma_start(out=tdb_s,
                              in_=mk_ap(tdb[:, :], 0, [[1, 128], [128, 4]]))
            lgb_s = wp.tile([128, 4], F32, tag="lgb")
            nc.sync.dma_start(out=lgb_s,
                              in_=mk_ap(lgb[:, :], 0, [[1, 128], [128, 4]]))
            bsel_s = wp.tile([128, 2, BSH], F32R, tag="bsel")
            nc.sync.dma_start(out=bsel_s,
                              in_=bsel[:, :].rearrange("(c p) j -> p c j", p=128))
            eye32_s = wp.tile([BSH, BSH], F32R, tag="eye32")
            nc.sync.dma_start(out=eye32_s, in_=eye32[:, :])
            atw_s = wp.tile([128, 4], BF16, tag="atw")
            nc.sync.dma_start(out=atw_s,
                              in_=mk_ap(atw[:, :], 0, [[1, 128], [128, 4]]))
            bdiag_s = wp.tile([128, BSH], BF16, tag="bdiag")
            nc.sync.dma_start(out=bdiag_s, in_=bdiag[:, :])
            t128_s = wp.tile([128, 128], BF16, tag="t128")
            nc.sync.dma_start(out=t128_s, in_=t128[:, :])
            ones_b = wp.tile([1, 384], BF16, tag="ones_b")
            nc.vector.memset(ones_b, 1.0)
            img_s = wp.tile([128, 4, RB], BF16, tag="img")
            favgp_s = wp.tile([128, 4, B], F32, tag="favgp")
            mask_all = wp.tile([128, T, B], U8, tag="maskall")
            mrow = masks[:, :]
            nc.sync.dma_start(
                out=mask_all,
                in_=bass.AP(tensor=mrow.tensor, offset=mrow.offset,
                            ap=[[0, 128], [B, T], [1, B]]))
            outb_s = wp.tile([128, 10], F32, tag="outb")
            nc.sync.dma_start(
                out=outb_s, in_=mk_ap(outb[:, :], 0, [[1, 128], [128, 10]]))

            h1_s = stp.tile([128, B], F32, tag="h1")
            c1_s = stp.tile([128, B], F32, tag="c1")
            h2_s = stp.tile([128, B], F32, tag="h2")
            c2_s = stp.tile([128, B], F32, tag="c2")
            for s in (h1_s, c1_s, h2_s, c2_s):
                nc.vector.memset(s, 0.0)
            h1g_s = stp.tile([128, 8, B], BF16, tag="h1g")
            h2g_a = stp.tile([128, 8, B], BF16, tag="h2ga")
            h2g_b = stp.tile([128, 8, B], BF16, tag="h2gb")
            h2g_bufs = [h2g_a, h2g_b]
            nc.vector.memset(h1g_s, 0.0)
            for hb in h2g_bufs:
                nc.vector.memset(hb, 0.0)

            # ================= phase 0 =================
            with (
                tc.tile_pool(name="p0", bufs=2) as p0,
                tc.tile_pool(name="p0w", bufs=1) as p0w,
                tc.tile_pool(name="p0ps", bufs=1, space="PSUM") as p0ps,
            ):
                wfavg_s = load_T(p0w, wfavg, F, GS, "wfavg")
                afb_t = p0.tile([1, A], BF16, tag="afb")
                nc.sync.dma_start(out=afb_t, in_=afb[:, :])
                # img_att = af_w @ fts^T + (af_b + ad_b); streamed over kt
                for nch in range(3):
                    ia_ps = p0ps.tile([128, 4, 512], F32, tag="iaps")
                    for kt in range(16):
                        afw_c = p0.tile([128, A], BF16, tag="afwc")
                        nc.sync.dma_start(
                            out=afw_c,
                            in_=mk_ap(afw[:, :], kt * 128 * A,
                                      [[A, 128], [1, A]]))
                        fts_c = p0.tile([128, 384], BF16, tag="ftsc")
                        nc.sync.dma_start(
                            out=fts_c,
                            in_=mk_ap(ftsT[:, :], kt * 128 * RB + nch * 384,
                                      [[RB, 128], [1, 384]]))
                        for m in range(4):
                            nc.tensor.matmul(
                                ia_ps[:, m, 0:384],
                                afw_c[:, m * 128:(m + 1) * 128],
                                fts_c[:, :], start=(kt == 0), stop=False)
                    for m in range(4):
                        nc.tensor.matmul(
                            ia_ps[:, m, 0:384], afb_t[:, m * 128:(m + 1) * 128],
                            ones_b[:, :], start=False, stop=True)
                        nc.scalar.copy(
                            img_s[:, m, nch * 384:(nch + 1) * 384],
                            ia_ps[:, m, 0:384])
                # favg for my batch shard (mean over regions)
                favg_t = p0.tile([128, 16, BSH], F32, tag="favg")
                for kt in range(16):
                    fts_f = p0.tile([128, RB], BF16, tag="ftsf")
                    nc.sync.dma_start(
                        out=fts_f,
                        in_=mk_ap(ftsT[:, :], kt * 128 * RB,
                                  [[RB, 128], [1, RB]]))
                    nc.vector.reduce_sum(
                        out=favg_t[:, kt, :],
                        in_=fts_f.rearrange("p (r b) -> p b r", r=R),
                        axis=mybir.AxisListType.X)
                favb = p0.tile([128, 16, BSH], BF16, tag="favb")
                nc.scalar.mul(favb, favg_t, 1.0 / R)
                nc.sync.dma_start(
                    out=mk_ap(agin_favg[:, :], 0,
                              [[BSH, 128], [128 * BSH, 16], [1, BSH]]),
                    in_=favb)
                ag(agin_favg[:, :], agout_favg[:, :, :])
                favgg = p0w.tile([128, 16, B], BF16, tag="favgg")
                for r in range(NC):
                    nc.sync.dma_start(
                        out=favgg[:, :, r * BSH:(r + 1) * BSH],
                        in_=mk_ap(agout_favg[:, :, :], r * F * BSH,
                                  [[BSH, 128], [128 * BSH, 16], [1, BSH]]))
                fp_ps = p0ps.tile([128, 4, 512], F32, tag="fpps")
                for kt in range(16):
                    for m in range(4):
                        nc.tensor.matmul(
                            fp_ps[:, m, 0:B],
                            wfavg_s[:, kt, m * 128:(m + 1) * 128],
                            favgg[:, kt, :], start=(kt == 0), stop=(kt == 15))
                for m in range(4):
                    nc.vector.tensor_scalar_add(favgp_s[:, m, :],
                                                fp_ps[:, m, 0:B],
                                                tdb_s[:, m:m + 1])

            # big loads needed from step 0 on, emitted after phase-0 streams
            adw_s = load_T(wp, adwT, D, A, "adw")
            ftsrb_s = load_T(wp, ftsrb, RB, F, "ftsrb")
            outw_s = load_T(wp, outw, D, VSH, "outw")

            # ================= phase 1: recurrence =================
            with (
                tc.tile_pool(name="p1", bufs=2) as p1,
                tc.tile_pool(name="pawg", bufs=1) as pawg,
                tc.tile_pool(name="pemb", bufs=2) as pemb,
                tc.tile_pool(name="p1c", bufs=1) as p1c,
                tc.tile_pool(name="psg", bufs=1, space="PSUM") as psg,
                tc.tile_pool(name="psa", bufs=2, space="PSUM") as psa,
            ):
                def vocab_block(tprev, h2g):
                    # vocab logits for step tprev from gathered h2(tprev);
                    # runs inside step tprev+1's h2-AllGather window
                    mkf = p1.tile([128, B], F32, tag="mkf")
                    mfr = maskf[:, :]
                    nc.sync.dma_start(
                        out=mkf,
                        in_=bass.AP(tensor=mfr.tensor,
                                    offset=mfr.offset + tprev * B,
                                    ap=[[0, 128], [1, B]]))
                    for half in range(5):
                        lp = psa.tile([128, 2, B], F32, tag="small")
                        for mm in range(2):
                            m = half * 2 + mm
                            for kt in range(8):
                                nc.tensor.matmul(
                                    lp[:, mm, :],
                                    outw_s[:, kt, m * 128:(m + 1) * 128],
                                    h2g[:, kt, :],
                                    start=(kt == 0), stop=(kt == 7))
                        ls = p1c.tile([128, 2, B], F32, tag="vls")
                        for mm in range(2):
                            m = half * 2 + mm
                            nc.vector.tensor_scalar_add(
                                ls[:, mm, :], lp[:, mm, :], outb_s[:, m:m + 1])
                        nc.vector.tensor_mul(ls, ls, bcast_r(mkf[:, :], 2))
                        nc.sync.dma_start(
                            out=mk_ap(outp[:, :],
                                      half * 2 * 128 * TB + tprev * B,
                                      [[TB, 128], [128 * TB, 2], [1, B]]),
                            in_=ls)

                for t in range(nsteps):
                    h2g_prev = h2g_bufs[t % 2]
                    h2g_next = h2g_bufs[(t + 1) % 2]
                    emb_t = pemb.tile([128, 8, B], BF16, tag="embt")
                    nc.sync.dma_start(
                        out=emb_t,
                        in_=mk_ap(embsT[:, :], t * B,
                                  [[TB, 128], [128 * TB, 8], [1, B]]))
                    mask_t = mask_all[:, t, :]

                    # ---- td-LSTM gates (m-interleaved, bank-exclusive) ----
                    # emb part first (no deps), h2g part last (arrives last)
                    tdps = psg.tile([128, 4, 512], F32, tag="gps")
                    for m in range(4):
                        sl = slice(m * 128, (m + 1) * 128)
                        for kt in range(8):
                            nc.tensor.matmul(tdps[:, m, 0:B], we_s[:, kt, sl],
                                             emb_t[:, kt, :],
                                             start=(kt == 0), stop=False)
                        for kt in range(8):
                            nc.tensor.matmul(tdps[:, m, 0:B],
                                             wtdh1_s[:, kt, sl],
                                             h1g_s[:, kt, :],
                                             start=False, stop=False)
                    for m in range(4):
                        sl = slice(m * 128, (m + 1) * 128)
                        for kt in range(8):
                            nc.tensor.matmul(tdps[:, m, 0:B],
                                             wtdh2_s[:, kt, sl],
                                             h2g_prev[:, kt, :],
                                             start=False, stop=(kt == 7))
                    tdv = tdps[:, :, 0:B]
                    nc.vector.tensor_add(tdv, tdv, favgp_s)
                    gi = p1c.tile([128, B], F32, tag="g0")
                    nc.scalar.activation(gi, tdps[:, 0, 0:B], AF.Sigmoid)
                    gf = p1c.tile([128, B], F32, tag="g1")
                    nc.scalar.activation(gf, tdps[:, 1, 0:B], AF.Sigmoid)
                    gg = p1c.tile([128, B], F32, tag="g2")
                    nc.scalar.activation(gg, tdps[:, 2, 0:B], AF.Tanh)
                    go = p1c.tile([128, B], F32, tag="g3")
                    nc.scalar.activation(go, tdps[:, 3, 0:B], AF.Sigmoid)
                    t1 = p1c.tile([128, B], F32, tag="x")
                    nc.vector.tensor_mul(t1, gf, c1_s)
                    t2 = p1c.tile([128, B], F32, tag="y")
                    nc.vector.tensor_mul(t2, gi, gg)
                    c1n = p1c.tile([128, B], F32, tag="g0")
                    nc.vector.tensor_add(c1n, t1, t2)
                    tc1 = p1c.tile([128, B], F32, tag="g1")
                    nc.scalar.activation(tc1, c1n, AF.Tanh)
                    h1n = p1c.tile([128, B], F32, tag="g2")
                    nc.vector.tensor_mul(h1n, go, tc1)
                    nc.vector.copy_predicated(c1_s, mask_t, c1n)
                    nc.vector.copy_predicated(h1_s, mask_t, h1n)

                    # ---- h1 allgather (bf16) ----
                    h1b = p1c.tile([128, B], BF16, tag="h1b")
                    nc.vector.tensor_copy(h1b, h1_s)
                    nc.sync.dma_start(out=agin_h1[:, :], in_=h1b)
                    ag(agin_h1[:, :], agout_h1[:, :, :])
                    nc.sync.dma_start(
                        out=h1g_s,
                        in_=agout_h1[:, :, :].rearrange("r p b -> p r b"))

                    # ---- dec_att via transposed GEMM + input-driven select
                    dtp = psg.tile([128, 2, 512], F32, tag="gps")
                    for kt in range(8):
                        for bc in range(2):
                            nc.tensor.matmul(
                                dtp[:, bc, :],
                                h1g_s[:, kt, bc * 128:(bc + 1) * 128],
                                adw_s[:, kt, :], start=(kt == 0), stop=(kt == 7))
                    decT_sb = p1c.tile([128, 2, A], F32R, tag="decT")
                    nc.scalar.copy(decT_sb, dtp)
                    dsp = psa.tile([BSH, A], F32, tag="small")
                    for bc in range(2):
                        nc.tensor.matmul(dsp[:, :], bsel_s[:, bc, :],
                                         decT_sb[:, bc, :],
                                         start=(bc == 0), stop=(bc == 1))
                    dsel_sb = p1c.tile([BSH, A], F32R, tag="dsel")
                    nc.scalar.copy(dsel_sb, dsp)
                    ttp = psa.tile([128, 4, BSH], F32, tag="small")
                    for q in range(4):
                        nc.tensor.transpose(
                            ttp[:, q, :].bitcast(F32R),
                            dsel_sb[:, q * 128:(q + 1) * 128],
                            eye32_s[:, :])
                    dec = p1c.tile([128, 4, BSH], BF16, tag="dec")
                    nc.scalar.copy(dec, ttp)

                    # ---- attention ----
                    scps = psa.tile([128, 40], F32, tag="small")
                    for m in range(4):
                        rel = p1c.tile([128, RB], BF16, tag="rel")
                        nc.vector.tensor_add(
                            rel.rearrange("p (r b) -> p r b", r=R),
                            img_s[:, m, :].rearrange("p (r b) -> p r b", r=R),
                            bcast_r(dec[:, m, :], R))
                        nc.scalar.activation(rel, rel, AF.Relu)
                        for c in range(9):
                            nc.tensor.matmul(
                                scps[:, m * 9 + c:m * 9 + c + 1],
                                rel[:, c * 128:(c + 1) * 128],
                                atw_s[:, m:m + 1],
                                start=True, stop=True)
                    scs = p1c.tile([128, 9], F32, tag="scs")
                    nc.vector.reduce_sum(
                        out=scs,
                        in_=scps[:, 0:36].rearrange("p (m c) -> p c m", m=4),
                        axis=mybir.AxisListType.X)
                    exps = p1c.tile([128, 9], F32, tag="exps")
                    nc.scalar.activation(exps, scs, AF.Exp)
                    expb = p1c.tile([128, 9], BF16, tag="expb")
                    nc.vector.tensor_copy(expb, exps)
                    for c in range(9):
                        nc.tensor.matmul(scps[:, 36:37], t128_s[:, :],
                                         expb[:, c:c + 1],
                                         start=(c == 0), stop=(c == 8))
                    rinv = p1c.tile([128, 1], F32, tag="rinv")
                    nc.vector.reciprocal(rinv, scps[:, 36:37])
                    arhs = p1c.tile([128, 9, BSH], BF16, tag="arhs")
                    for c in range(9):
                        nc.vector.tensor_scalar(
                            arhs[:, c, :], bdiag_s, exps[:, c:c + 1],
                            rinv[:, 0:1], mybir.AluOpType.mult,
                            mybir.AluOpType.mult)
                    # ---- aw einsum (ftsrb SBUF-resident) ----
                    awacc = p1c.tile([128, 16, BSH], F32, tag="awsb")
                    for c in range(9):
                        awps = psa.tile([128, 16, BSH], F32, tag="awps")
                        for fc in range(16):
                            nc.tensor.matmul(
                                awps[:, fc, :],
                                ftsrb_s[:, c, fc * 128:(fc + 1) * 128],
                                arhs[:, c, :],
                                start=True, stop=True)
                        if c == 0:
                            nc.vector.tensor_copy(awacc, awps)
                        else:
                            nc.vector.tensor_add(awacc, awacc, awps)
                    awb = p1c.tile([128, 16, BSH], BF16, tag="awb")
                    nc.vector.tensor_copy(awb, awacc)
                    nc.sync.dma_start(
                        out=mk_ap(agin_aw[:, :], 0,
                                  [[BSH, 128], [128 * BSH, 16], [1, BSH]]),
                        in_=awb)
                    ag(agin_aw[:, :], agout_aw[:, :, :])
                    awg = pawg.tile([128, 16, B], BF16, tag="awg")
                    for r in range(NC):
                        nc.sync.dma_start(
                            out=awg[:, :, r * BSH:(r + 1) * BSH],
                            in_=mk_ap(agout_aw[:, :, :], r * F * BSH,
                                      [[BSH, 128], [128 * BSH, 16], [1, BSH]]))

                    # ---- lg-LSTM gates ----
                    lgps = psg.tile([128, 4, 512], F32, tag="gps")
                    for m in range(4):
                        sl = slice(m * 128, (m + 1) * 128)
                        for kt in range(8):
                            nc.tensor.matmul(lgps[:, m, 0:B],
                                             wlgh1_s[:, kt, sl],
                                             h1g_s[:, kt, :],
                                             start=(kt == 0), stop=False)
                        for kt in range(8):
                            nc.tensor.matmul(lgps[:, m, 0:B],
                                             wlgh2_s[:, kt, sl],
                                             h2g_prev[:, kt, :],
                                             start=False, stop=False)
                    for m in range(4):
                        sl = slice(m * 128, (m + 1) * 128)
                        for kt in range(16):
                            nc.tensor.matmul(lgps[:, m, 0:B],
                                             wlgaw_s[:, kt, sl],
                                             awg[:, kt, :],
                                             start=False, stop=(kt == 15))
                    gi2 = p1c.tile([128, B], F32, tag="g0")
                    nc.scalar.activation(gi2, lgps[:, 0, 0:B], AF.Sigmoid,
                                         bias=lgb_s[:, 0:1])
                    gf2 = p1c.tile([128, B], F32, tag="g1")
                    nc.scalar.activation(gf2, lgps[:, 1, 0:B], AF.Sigmoid,
                                         bias=lgb_s[:, 1:2])
                    gg2 = p1c.tile([128, B], F32, tag="g2")
                    nc.scalar.activation(gg2, lgps[:, 2, 0:B], AF.Tanh,
                                         bias=lgb_s[:, 2:3])
                    go2 = p1c.tile([128, B], F32, tag="g3")
                    nc.scalar.activation(go2, lgps[:, 3, 0:B], AF.Sigmoid,
                                         bias=lgb_s[:, 3:4])
                    t3 = p1c.tile([128, B], F32, tag="x")
                    nc.vector.tensor_mul(t3, gf2, c2_s)
                    t4 = p1c.tile([128, B], F32, tag="y")
                    nc.vector.tensor_mul(t4, gi2, gg2)
                    c2n = p1c.tile([128, B], F32, tag="g0")
                    nc.vector.tensor_add(c2n, t3, t4)
                    tc2 = p1c.tile([128, B], F32, tag="g1")
                    nc.scalar.activation(tc2, c2n, AF.Tanh)
                    h2n = p1c.tile([128, B], F32, tag="g2")
                    nc.vector.tensor_mul(h2n, go2, tc2)
                    nc.vector.copy_predicated(c2_s, mask_t, c2n)
                    nc.vector.copy_predicated(h2_s, mask_t, h2n)

                    # ---- h2 allgather (bf16); vocab(t-1) fills the window
                    h2b = p1c.tile([128, B], BF16, tag="h2b")
                    nc.vector.tensor_copy(h2b, h2_s)
                    nc.sync.dma_start(out=agin_h2[:, :], in_=h2b)
                    ag(agin_h2[:, :], agout_h2[:, :, :])
                    if t > 0:
                        vocab_block(t - 1, h2g_prev)
                    nc.sync.dma_start(
                        out=h2g_next,
                        in_=agout_h2[:, :, :].rearrange("r p b -> p r b"))

                # tail: vocab for the last step
                vocab_block(nsteps - 1, h2g_bufs[nsteps % 2])

    nc.compile()
    return nc


def _build_cached():
    if "nc" not in _CACHED:
        _CACHED["nc"] = build()
    return _CACHED["nc"]


def host_prep(feats, sequences, sizes, emb, td_wih, td_whh, td_b,
              lg_wih, lg_whh, lg_b, af_w, af_b, ad_w, ad_b, at_w, at_b,
              out_w, out_b):
    f32 = np.float32
    bf = ml_dtypes.bfloat16
    lens = np.asarray(sizes).astype(np.int64)[:, 0]
    order = np.argsort(-lens, kind="stable")
    lens_s = lens[order]
    seq = np.asarray(sequences).astype(np.int64)[order]
    fts = np.ascontiguousarray(np.asarray(feats, f32)[order])

    embs = np.asarray(emb, f32)[seq[:, :T]]
    embsT = np.ascontiguousarray(embs.transpose(2, 1, 0)).reshape(E, TB)

    mask = (np.arange(T)[None, :] < (lens_s - 1)[:, None])
    masks = np.ascontiguousarray(mask.T).astype(np.uint8)
    maskf = np.ascontiguousarray(mask.T.reshape(TB, 1)).astype(f32)

    bdiag = np.tile(np.eye(BSH, dtype=f32), (4, 1)).astype(bf)
    t128 = np.tile(np.eye(BSH, dtype=f32), (4, 4)).astype(bf)

    td_wih = np.asarray(td_wih, f32)
    td_whh = np.asarray(td_whh, f32)
    lg_wih = np.asarray(lg_wih, f32)
    lg_whh = np.asarray(lg_whh, f32)
    af_wT = np.ascontiguousarray(np.asarray(af_w, f32).T).astype(bf)
    afb_full = np.asarray(af_b, f32) + np.asarray(ad_b, f32)
    ad_wv = np.asarray(ad_w, f32)
    adwT_full = np.ascontiguousarray(ad_wv.T).astype(bf)
    eye32_np = np.eye(BSH, dtype=f32)

    def bsel_k(k):
        m = np.zeros((B, BSH), f32)
        m[np.arange(k * BSH, (k + 1) * BSH), np.arange(BSH)] = 1.0
        return m
    atwT = np.ascontiguousarray(np.asarray(at_w, f32).T).astype(bf)
    out_wv = np.asarray(out_w, f32)
    out_bv = np.asarray(out_b, f32)

    in_maps = []
    for k in range(NC):
        gsl = np.concatenate([np.arange(g * D + k * 128, g * D + (k + 1) * 128)
                              for g in range(4)])
        bsl = slice(k * BSH, (k + 1) * BSH)
        fsh = fts[bsl]
        ftsT_k = np.ascontiguousarray(
            fsh.transpose(2, 1, 0).reshape(F, RB)).astype(bf)
        ftsrb_k = np.ascontiguousarray(
            fsh.transpose(1, 0, 2).reshape(RB, F)).astype(bf)
        ow_pad = np.zeros((VSH, D), f32)
        ow_pad[:1250] = out_wv[k * 1250:(k + 1) * 1250]
        ob_pad = np.zeros((VSH, 1), f32)
        ob_pad[:1250, 0] = out_bv[k * 1250:(k + 1) * 1250]
        in_maps.append({
            "wtdh2": np.ascontiguousarray(td_wih[gsl, 0:D].T).astype(bf),
            "wtdh1": np.ascontiguousarray(td_whh[gsl].T).astype(bf),
            "wfavg": np.ascontiguousarray(td_wih[gsl, D:D + F].T).astype(bf),
            "we": np.ascontiguousarray(td_wih[gsl, D + F:].T).astype(bf),
            "tdb": np.ascontiguousarray(np.asarray(td_b, f32)[gsl][None, :]),
            "wlgaw": np.ascontiguousarray(lg_wih[gsl, 0:F].T).astype(bf),
            "wlgh1": np.ascontiguousarray(lg_wih[gsl, F:].T).astype(bf),
            "wlgh2": np.ascontiguousarray(lg_whh[gsl].T).astype(bf),
            "lgb": np.ascontiguousarray(np.asarray(lg_b, f32)[gsl][None, :]),
            "adwT": adwT_full,
            "bsel": bsel_k(k),
            "eye32": eye32_np,
            "afw": af_wT,
            "afb": np.ascontiguousarray(afb_full[None, :]).astype(bf),
            "atw": atwT,
            "ftsT": ftsT_k,
            "ftsrb": ftsrb_k,
            "embsT": embsT.astype(bf),
            "outw": np.ascontiguousarray(ow_pad.T).astype(bf),
            "outb": ob_pad,
            "masks": masks,
            "maskf": maskf,
            "bdiag": bdiag,
            "t128": t128,
        })
    return in_maps


def kernel(**inputs):
    in_maps = host_prep(**inputs)
    nc = _build_cached()
    res = run_bass_kernel_spmd(nc, in_maps, core_ids=list(range(NC)))
    shards = [res.results[k]["outp"].reshape(VSH, T, B)[:1250]
              for k in range(NC)]
    full = np.concatenate(shards, axis=0)
    return np.ascontiguousarray(full.transpose(2, 1, 0))
